# revision 15
# baseline (speedup 1.0000x reference)
"""Binary successive-approximation encoder on 8 Trainium2 NeuronCores.

Full input x [16, 1024, 512] f32 -> output [16, 1024, n_bits, 512] f32.

Math: for y in [0, 1) on the 2^-23 grid (jax uniform f32), plane k
(MSB first) is bit (n_bits-1-k) of floor(y * 2^n_bits).

v8 pipeline, per 256-row tile (J=2 consecutive rows per partition):
  ACT  : yi = u16(round(x*2^n_bits - (0.5 - 2^(n_bits-24))))
         == floor(x*2^n_bits) EXACTLY: the bias shifts every grid point
         strictly inside a round-to-nearest window (never a tie), and
         the f32 mult/sub are exact on the grid (24-bit span).
  DVE  : plane k = (yi >> (n_bits-1-k)) & 1, one fused u16 bitvec
         tensor_scalar per plane (u16 keeps the DVE 16-bit 4x fast
         path; bitvec cannot cast and f32/i32 run at 1x).
  u16 planes then leave through THREE parallel cast channels (the u8
  compaction is the bottleneck -- the SWDGE cce cast datapath alone
  runs at only ~250 GB/s):
    planes [0, SW)        : SWDGE casting DMA u16 SBUF -> u8 HBM (Pool
                            issues; only the software DGE can cast)
    planes [SW, SW+PL)    : Pool tensor_copy cast -> u8 staging
    planes [SW+PL, n)     : ACT copy cast -> u8 staging
  SP   : input DMAs (prefetched) + plain HWDGE u8 DMAs of the staging.
The host upcasts u8 -> f32 at gather (exact: values are 0/1).

Row mapping r = p*(TILES*J) + t*J + j keeps every DMA contiguous per
partition (SWDGE descriptors are generated in software on the Q7; a
fragmented pattern multiplies their count).

Why not f32 planes (the old baseline): 4x the HBM write traffic plus a
second full DVE convert pass -- that kernel was simultaneously
DVE-bound (125us busy) and DMA-bound (113us) at 124us total.

Sharding: batch dim 16 -> 8 cores x 2 batches, no communication.

This walrus build allows only ONE sync wait per instruction, hence
_SplitDrainTileContext: every scheduled instruction with N>1 waits gets
N-1 preceding same-engine no-ops carrying one wait each, and the tail
drain's aggregated waits ride on SP no-ops.
"""

import numpy as np

import concourse.bass as bass
import concourse.mybir as mybir
import concourse.tile as tile
from concourse.bass_utils import run_bass_kernel_spmd

B, T, C = 16, 1024, 512
N_CORES = 8
P = 128                       # SBUF partitions
ROWS = B * T // N_CORES       # 2048 (b,t) rows per core
TILES = 8
J = ROWS // (P * TILES)       # 2 consecutive rows per partition per tile

_nc_cache: dict[int, bass.Bass] = {}


class _SplitDrainTileContext(tile.TileContext):
    """TileContext for a walrus build that rejects multi-wait instructions
    ("Too many sync wait commands", one sync wait allowed per instruction):
    every scheduled instruction with N>1 waits is preceded by N-1 same-engine
    no-ops carrying one wait each (same-engine in-order execution makes this
    equivalent), and the tail drain's aggregated waits ride on SP no-ops."""

    def _add_instruction(self, inst):
        si = inst.sync_info
        if (
            si is not None
            and si.on_wait
            and len(si.on_wait) > 1
            and inst.engine != mybir.EngineType.Unassigned
        ):
            waits = list(si.on_wait)
            si.on_wait = waits[-1:]
            for w in waits[:-1]:
                nop = mybir.InstNoOp(
                    name=self.nc.get_next_instruction_name(),
                    sync_info=mybir.SyncInfo(on_wait=[w], on_update=[]),
                    bass_nofuse=True,
                    engine=inst.engine,
                )
                super()._add_instruction(nop)
        super()._add_instruction(inst)

    def _drain_and_barrier(self, tick_clock, wait_clock):
        import bass_rust
        from concourse.vector_clock import ScopedClock

        nc = self.nc
        drain_inst = nc.sync.drain()
        wait_clock.add_sem_waits(
            drain_inst.ins, ScopedClock({None: tick_clock.global_clock})
        )
        si = drain_inst.ins.sync_info
        waits = list(si.on_wait) if si is not None else []
        if len(waits) > 1:
            si.on_wait = waits[:1]
            for w in waits[1:]:
                nop = nc.sync.nop()
                nop.ins.sync_info = bass_rust.SyncInfo(on_wait=[w], on_update=[])
        nc.all_engine_barrier()
        assert self.sems is not None
        popped = nc._tile_sem_poison_stack.pop()
        assert popped is self._sem_poison
        nc.clear_and_free_semaphores(list(self.sems.allocated().values()))
        nc.all_engine_barrier()


def _build(n_bits: int) -> bass.Bass:
    if n_bits in _nc_cache:
        return _nc_cache[n_bits]
    A = mybir.AluOpType
    f32, u16, u8 = mybir.dt.float32, mybir.dt.uint16, mybir.dt.uint8
    KC = n_bits * C
    # u16 planes + exact-floor bias both need n_bits <= 15
    assert 1 <= n_bits <= 15
    SCALE = float(2**n_bits)
    FLOOR_BIAS = -(0.5 - 2.0 ** (n_bits - 24))
    JC = J * C
    # cast-channel split: SWDGE / Pool tensor_copy / ACT copy
    SW = max(1, round(n_bits * 0.6))
    PL = (n_bits - SW + 1) // 2
    AC = n_bits - SW - PL
    assert SW + PL + AC == n_bits and PL >= 0 and AC >= 0

    nc = bass.Bass(
        "TRN2", target_bir_lowering=False, debug=False, num_swdge_queues=2
    )
    x = nc.dram_tensor("x", [ROWS, C], f32, kind="ExternalInput")
    out = nc.dram_tensor("out", [ROWS, KC], u8, kind="ExternalOutput")
    # prewarm target mimicking the real SWDGE pattern: J runs of SW*C
    # bytes per partition with a row-stride gap
    warm = nc.dram_tensor("warm", [P, J, KC], u8, kind="Internal")
    # row r = p*(TILES*J) + t*J + j
    xr = x.ap().rearrange("(p t j) c -> t p (j c)", p=P, t=TILES)
    orr = out.ap().rearrange("(p t j) kc -> t p j kc", p=P, t=TILES)

    with _SplitDrainTileContext(nc) as tc:
        with (
            tc.tile_pool(name="xin", bufs=TILES) as xin,
            tc.tile_pool(name="yint", bufs=3) as yip,
            tc.tile_pool(name="st16", bufs=4) as s16p,
            tc.tile_pool(name="st8", bufs=4) as s8p,
            tc.tile_pool(name="wrm", bufs=1) as wrmp,
        ):
            # all input DMAs first on the SP ring: they drain during the
            # compute ramp, so the steady state is pure output traffic
            xts = []
            for t in range(TILES):
                xt = xin.tile([P, JC], f32)
                nc.sync.dma_start(xt[:], xr[t])
                xts.append(xt)
            # prewarm the Q7 SWDGE big-transfer path on both queues: the
            # first DMA of a given shape class costs ~10us extra
            wt = wrmp.tile([P, J * SW * C], u16)
            nc.gpsimd.memset(wt[:], 0)
            for q in range(2):
                wi = nc.gpsimd.dma_start(
                    warm.ap()[:, :, : SW * C],
                    wt[:].rearrange("p (j sc) -> p j sc", j=J),
                )
                if q:
                    wi.ins.queue = "qPoolDynamic1"
            for t in range(TILES):
                xt = xts[t]
                yi = yip.tile([P, JC], u16)
                # yi = floor(x * 2^n_bits) on ACT (exact, see module doc)
                nc.scalar.activation(
                    yi[:], xt[:], mybir.ActivationFunctionType.Copy,
                    bias=FLOOR_BIAS, scale=SCALE,
                )
                yiv = yi[:].rearrange("p (j c) -> p j c", j=J)
                st = s16p.tile([P, J * KC], u16)
                sv = st[:].rearrange("p (j k c) -> p j k c", j=J, k=n_bits)
                for k in range(n_bits):
                    nc.vector.tensor_scalar(
                        sv[:, :, k, :], yiv, n_bits - 1 - k, 1,
                        A.logical_shift_right, A.bitwise_and,
                    )
                    if k == SW - 1:
                        # SWDGE casting DMA u16 -> u8 for planes [0, SW)
                        oi = nc.gpsimd.dma_start(
                            orr[t][:, :, : SW * C],
                            sv[:, :, :SW, :],
                        )
                        if t % 2:
                            oi.ins.queue = "qPoolDynamic1"
                # planes [SW, n): cast to u8 staging on Pool / ACT, then
                # one plain HWDGE DMA on the SP ring
                s8 = s8p.tile([P, J * (PL + AC) * C], u8)
                s8v = s8[:].rearrange(
                    "p (j k c) -> p j k c", j=J, k=PL + AC
                )
                if PL:
                    nc.gpsimd.tensor_copy(
                        s8v[:, :, :PL, :], sv[:, :, SW : SW + PL, :]
                    )
                if AC:
                    nc.scalar.copy(
                        s8v[:, :, PL:, :], sv[:, :, SW + PL :, :]
                    )
                nc.sync.dma_start(
                    orr[t][:, :, SW * C :],
                    s8v,
                )
    _nc_cache[n_bits] = nc
    return nc


def kernel(**inputs) -> np.ndarray:
    x = np.ascontiguousarray(np.asarray(inputs["x"], dtype=np.float32))
    n_bits = int(inputs["n_bits"])
    assert x.shape == (B, T, C), x.shape
    nc = _build(n_bits)
    xs = x.reshape(N_CORES, ROWS, C)
    in_maps = [{"x": xs[c]} for c in range(N_CORES)]
    res = run_bass_kernel_spmd(nc, in_maps, core_ids=list(range(N_CORES)))
    out = np.stack(
        [res.results[c]["out"] for c in range(N_CORES)], axis=0
    )  # [8, 2048, n_bits*512] u8
    return out.reshape(B, T, n_bits, C).astype(np.float32)


# revision 19
# speedup vs baseline: 1.2084x; 1.2084x over previous
"""Binary successive-approximation encoder on 8 Trainium2 NeuronCores.

Full input x [16, 1024, 512] f32 -> output [16, 1024, n_bits, 512] f32.

Math: for y in [0, 1) on the 2^-23 grid (jax uniform f32), plane k
(MSB first) is bit (n_bits-1-k) of floor(y * 2^n_bits).

v8 pipeline, per 256-row tile (J=2 consecutive rows per partition):
  ACT  : yi = u16(round(x*2^n_bits - (0.5 - 2^(n_bits-24))))
         == floor(x*2^n_bits) EXACTLY: the bias shifts every grid point
         strictly inside a round-to-nearest window (never a tie), and
         the f32 mult/sub are exact on the grid (24-bit span).
  DVE  : plane k = (yi >> (n_bits-1-k)) & 1, one fused u16 bitvec
         tensor_scalar per plane (u16 keeps the DVE 16-bit 4x fast
         path; bitvec cannot cast and f32/i32 run at 1x).
  u16 planes then leave through THREE parallel cast channels (the u8
  compaction is the bottleneck -- the SWDGE cce cast datapath alone
  runs at only ~250 GB/s):
    planes [0, SW)        : SWDGE casting DMA u16 SBUF -> u8 HBM (Pool
                            issues; only the software DGE can cast)
    planes [SW, SW+PL)    : Pool tensor_copy cast -> u8 staging
    planes [SW+PL, n)     : ACT copy cast -> u8 staging
  SP   : input DMAs (prefetched) + plain HWDGE u8 DMAs of the staging.
The host upcasts u8 -> f32 at gather (exact: values are 0/1).

Row mapping r = p*(TILES*J) + t*J + j keeps every DMA contiguous per
partition (SWDGE descriptors are generated in software on the Q7; a
fragmented pattern multiplies their count).

Why not f32 planes (the old baseline): 4x the HBM write traffic plus a
second full DVE convert pass -- that kernel was simultaneously
DVE-bound (125us busy) and DMA-bound (113us) at 124us total.

Sharding: batch dim 16 -> 8 cores x 2 batches, no communication.

This walrus build allows only ONE sync wait per instruction, hence
_SplitDrainTileContext: every scheduled instruction with N>1 waits gets
N-1 preceding same-engine no-ops carrying one wait each, and the tail
drain's aggregated waits ride on SP no-ops.
"""

import numpy as np

import concourse.bass as bass
import concourse.mybir as mybir
import concourse.tile as tile
from concourse.bass_utils import run_bass_kernel_spmd

B, T, C = 16, 1024, 512
N_CORES = 8
P = 128                       # SBUF partitions
ROWS = B * T // N_CORES       # 2048 (b,t) rows per core
TILES = 8
J = ROWS // (P * TILES)       # 2 consecutive rows per partition per tile

_nc_cache: dict[int, bass.Bass] = {}


class _SplitDrainTileContext(tile.TileContext):
    """TileContext for a walrus build that rejects multi-wait instructions
    ("Too many sync wait commands", one sync wait allowed per instruction):
    every scheduled instruction with N>1 waits is preceded by N-1 same-engine
    no-ops carrying one wait each (same-engine in-order execution makes this
    equivalent), and the tail drain's aggregated waits ride on SP no-ops."""

    def _add_instruction(self, inst):
        si = inst.sync_info
        if (
            si is not None
            and si.on_wait
            and len(si.on_wait) > 1
            and inst.engine != mybir.EngineType.Unassigned
        ):
            waits = list(si.on_wait)
            si.on_wait = waits[-1:]
            for w in waits[:-1]:
                nop = mybir.InstNoOp(
                    name=self.nc.get_next_instruction_name(),
                    sync_info=mybir.SyncInfo(on_wait=[w], on_update=[]),
                    bass_nofuse=True,
                    engine=inst.engine,
                )
                super()._add_instruction(nop)
        super()._add_instruction(inst)

    def _drain_and_barrier(self, tick_clock, wait_clock):
        import bass_rust
        from concourse.vector_clock import ScopedClock

        nc = self.nc
        drain_inst = nc.sync.drain()
        wait_clock.add_sem_waits(
            drain_inst.ins, ScopedClock({None: tick_clock.global_clock})
        )
        si = drain_inst.ins.sync_info
        waits = list(si.on_wait) if si is not None else []
        if len(waits) > 1:
            si.on_wait = waits[:1]
            for w in waits[1:]:
                nop = nc.sync.nop()
                nop.ins.sync_info = bass_rust.SyncInfo(on_wait=[w], on_update=[])
        nc.all_engine_barrier()
        assert self.sems is not None
        popped = nc._tile_sem_poison_stack.pop()
        assert popped is self._sem_poison
        nc.clear_and_free_semaphores(list(self.sems.allocated().values()))
        nc.all_engine_barrier()


def _build(n_bits: int) -> bass.Bass:
    if n_bits in _nc_cache:
        return _nc_cache[n_bits]
    A = mybir.AluOpType
    f32, u16, u8 = mybir.dt.float32, mybir.dt.uint16, mybir.dt.uint8
    KC = n_bits * C
    # u16 planes + exact-floor bias both need n_bits <= 15
    assert 1 <= n_bits <= 15
    SCALE = float(2**n_bits)
    FLOOR_BIAS = -(0.5 - 2.0 ** (n_bits - 24))
    JC = J * C
    # cast-channel split: ACT copy-casts the first AC planes, the SWDGE
    # casting DMA handles the rest (Pool's own Cast op measured ~3.4
    # ns/elem -- useless)
    AC = 1 if n_bits >= 4 else 0
    SW = n_bits - AC

    nc = bass.Bass(
        "TRN2", target_bir_lowering=False, debug=False, num_swdge_queues=2
    )
    x = nc.dram_tensor("x", [ROWS, C], f32, kind="ExternalInput")
    out = nc.dram_tensor("out", [ROWS, KC], u8, kind="ExternalOutput")
    # prewarm target mimicking the real SWDGE pattern: J runs of SW*C
    # bytes per partition with a row-stride gap
    warm = nc.dram_tensor("warm", [P, J, KC], u8, kind="Internal")
    # row r = p*(TILES*J) + t*J + j
    xr = x.ap().rearrange("(p t j) c -> t p (j c)", p=P, t=TILES)
    orr = out.ap().rearrange("(p t j) kc -> t p j kc", p=P, t=TILES)

    with _SplitDrainTileContext(nc) as tc:
        with (
            tc.tile_pool(name="xin", bufs=TILES) as xin,
            tc.tile_pool(name="yint", bufs=3) as yip,
            tc.tile_pool(name="st16", bufs=5) as s16p,
            tc.tile_pool(name="st8", bufs=4) as s8p,
            tc.tile_pool(name="wrm", bufs=1) as wrmp,
        ):
            # all input DMAs first on the SP ring: they drain during the
            # compute ramp, so the steady state is pure output traffic
            xts = []
            for t in range(TILES):
                xt = xin.tile([P, JC], f32)
                nc.sync.dma_start(xt[:], xr[t])
                xts.append(xt)
            # prewarm the Q7 SWDGE big-transfer path on both queues: the
            # first DMA of a given shape class costs ~10us extra
            wt = wrmp.tile([P, J * SW * C], u16)
            nc.gpsimd.memset(wt[:], 0)
            for q in range(2):
                wi = nc.gpsimd.dma_start(
                    warm.ap()[:, :, : SW * C],
                    wt[:].rearrange("p (j sc) -> p j sc", j=J),
                )
                if q:
                    wi.ins.queue = "qPoolDynamic1"
            del wt
            for t in range(TILES):
                xt = xts[t]
                yi = yip.tile([P, JC], u16)
                # yi = floor(x * 2^n_bits) on ACT (exact, see module doc)
                nc.scalar.activation(
                    yi[:], xt[:], mybir.ActivationFunctionType.Copy,
                    bias=FLOOR_BIAS, scale=SCALE,
                )
                yiv = yi[:].rearrange("p (j c) -> p j c", j=J)
                st = s16p.tile([P, J * KC], u16)
                sv = st[:].rearrange("p (j k c) -> p j k c", j=J, k=n_bits)
                # plane 0 first: ACT casts + SP drains it while DVE
                # extracts the SWDGE planes
                for k in range(n_bits):
                    nc.vector.tensor_scalar(
                        sv[:, :, k, :], yiv, n_bits - 1 - k, 1,
                        A.logical_shift_right, A.bitwise_and,
                    )
                    if AC and k == AC - 1:
                        s8 = s8p.tile([P, J * AC * C], u8)
                        s8v = s8[:].rearrange(
                            "p (j k c) -> p j k c", j=J, k=AC
                        )
                        nc.scalar.copy(s8v, sv[:, :, :AC, :])
                        nc.sync.dma_start(orr[t][:, :, : AC * C], s8v)
                # SWDGE casting DMA u16 -> u8 for planes [AC, n_bits)
                oi = nc.gpsimd.dma_start(
                    orr[t][:, :, AC * C :], sv[:, :, AC:, :]
                )
                if t % 2:
                    oi.ins.queue = "qPoolDynamic1"
    _nc_cache[n_bits] = nc
    return nc


def kernel(**inputs) -> np.ndarray:
    x = np.ascontiguousarray(np.asarray(inputs["x"], dtype=np.float32))
    n_bits = int(inputs["n_bits"])
    assert x.shape == (B, T, C), x.shape
    nc = _build(n_bits)
    xs = x.reshape(N_CORES, ROWS, C)
    in_maps = [{"x": xs[c]} for c in range(N_CORES)]
    res = run_bass_kernel_spmd(nc, in_maps, core_ids=list(range(N_CORES)))
    out = np.stack(
        [res.results[c]["out"] for c in range(N_CORES)], axis=0
    )  # [8, 2048, n_bits*512] u8
    return out.reshape(B, T, n_bits, C).astype(np.float32)


# revision 37
# speedup vs baseline: 1.3386x; 1.1078x over previous
"""Binary successive-approximation encoder on 8 Trainium2 NeuronCores.

Full input x [16, 1024, 512] f32 -> output [16, 1024, n_bits, 512] f32.

Math: for y in [0, 1) on the 2^-23 grid (jax uniform f32), plane k
(MSB first) is bit (n_bits-1-k) of floor(y * 2^n_bits).

Pipeline, per 256-row tile (J=2 consecutive rows per partition):
  ACT  : yi = u16(round(x*2^n_bits - (0.5 - 2^(n_bits-24))))
         == floor(x*2^n_bits) EXACTLY: the bias shifts every grid point
         strictly inside a round-to-nearest window (never a tie), and
         the f32 mult/sub are exact on the grid (24-bit span).
  DVE  : plane k = (yi >> (n_bits-1-k)) & 1, one fused u16 bitvec
         tensor_scalar per plane. u16 keeps the DVE 16-bit fast path
         (0.4 ns/lane-elem measured; f32/i32 run at 1x and bitvec
         cannot cast to u8 anyway -- the verifier rejects it).
  Pool : SWDGE casting DMAs u16 SBUF -> u8 HBM (only the software DGE
         can cast; HWDGE cannot). Each tile drains as TWO half-plane
         casts from independent tile pools so the first DMA fires while
         DVE is still extracting the second half, and buffer recycle is
         half-tile granular. HBM write traffic is 1 byte per output
         element (10.5 MB/core instead of the f32 baseline's 40 MB).
  SP   : input DMAs, all prefetched upfront on the HWDGE ring.
The host upcasts u8 -> f32 at gather (exact: values are 0/1).

Row mapping r = p*(TILES*J) + t*J + j keeps every DMA contiguous per
partition (SWDGE descriptors are generated in software on the Q7; a
fragmented pattern multiplies their count and cost).

The first SWDGE DMA of a given size class pays a one-time ~10us ucode
cost, so a full-size prewarm DMA to a scratch tensor runs on each SWDGE
queue during the input ramp.

Measured landscape (per core): DVE extraction ~31us, SWDGE cast channel
~3us/half-tile marginal, ACT scales ~9us, HBM traffic 14.5 MB. The f32
baseline (extract planes to i32, convert to f32, write 44 MB) was
simultaneously DVE-bound (125us busy) and DMA-bound (113us) at 124us;
this version benches ~80us with the cast channel as the critical path.
Rejected alternatives: Pool tensor_copy cast (3.4 ns/elem, too slow),
ACT copy cast (1 ns/elem, can only absorb ~1 plane), mod/divide-based
arith extraction (ops banned in tensor_scalar), u16 HWDGE tail (extra
HBM bytes outweigh the SWDGE relief).

Sharding: batch dim 16 -> 8 cores x 2 batches, no communication.

This walrus build allows only ONE sync wait per instruction, hence
_SplitDrainTileContext: every scheduled instruction with N>1 waits gets
N-1 preceding same-engine no-ops carrying one wait each, and the tail
drain's aggregated waits ride on SP no-ops.
"""

import numpy as np

import concourse.bass as bass
import concourse.mybir as mybir
import concourse.tile as tile
from concourse.bass_utils import run_bass_kernel_spmd

B, T, C = 16, 1024, 512
N_CORES = 8
P = 128                       # SBUF partitions
ROWS = B * T // N_CORES       # 2048 (b,t) rows per core
TILES = 8
J = ROWS // (P * TILES)       # 2 consecutive rows per partition per tile

_nc_cache: dict[int, bass.Bass] = {}


class _SplitDrainTileContext(tile.TileContext):
    """TileContext for a walrus build that rejects multi-wait instructions
    ("Too many sync wait commands", one sync wait allowed per instruction):
    every scheduled instruction with N>1 waits is preceded by N-1 same-engine
    no-ops carrying one wait each (same-engine in-order execution makes this
    equivalent), and the tail drain's aggregated waits ride on SP no-ops."""

    def _add_instruction(self, inst):
        si = inst.sync_info
        if (
            si is not None
            and si.on_wait
            and len(si.on_wait) > 1
            and inst.engine != mybir.EngineType.Unassigned
        ):
            waits = list(si.on_wait)
            si.on_wait = waits[-1:]
            for w in waits[:-1]:
                nop = mybir.InstNoOp(
                    name=self.nc.get_next_instruction_name(),
                    sync_info=mybir.SyncInfo(on_wait=[w], on_update=[]),
                    bass_nofuse=True,
                    engine=inst.engine,
                )
                super()._add_instruction(nop)
        super()._add_instruction(inst)

    def _drain_and_barrier(self, tick_clock, wait_clock):
        import bass_rust
        from concourse.vector_clock import ScopedClock

        nc = self.nc
        drain_inst = nc.sync.drain()
        wait_clock.add_sem_waits(
            drain_inst.ins, ScopedClock({None: tick_clock.global_clock})
        )
        si = drain_inst.ins.sync_info
        waits = list(si.on_wait) if si is not None else []
        if len(waits) > 1:
            si.on_wait = waits[:1]
            for w in waits[1:]:
                nop = nc.sync.nop()
                nop.ins.sync_info = bass_rust.SyncInfo(on_wait=[w], on_update=[])
        nc.all_engine_barrier()
        assert self.sems is not None
        popped = nc._tile_sem_poison_stack.pop()
        assert popped is self._sem_poison
        nc.clear_and_free_semaphores(list(self.sems.allocated().values()))
        nc.all_engine_barrier()


def _build(n_bits: int) -> bass.Bass:
    if n_bits in _nc_cache:
        return _nc_cache[n_bits]
    A = mybir.AluOpType
    f32, u16, u8 = mybir.dt.float32, mybir.dt.uint16, mybir.dt.uint8
    KC = n_bits * C
    # u16 planes + exact-floor bias both need n_bits <= 15
    assert 1 <= n_bits <= 15
    SCALE = float(2**n_bits)
    FLOOR_BIAS = -(0.5 - 2.0 ** (n_bits - 24))
    JC = J * C

    nc = bass.Bass(
        "TRN2", target_bir_lowering=False, debug=False, num_swdge_queues=2
    )
    x = nc.dram_tensor("x", [ROWS, C], f32, kind="ExternalInput")
    out = nc.dram_tensor("out", [ROWS, KC], u8, kind="ExternalOutput")
    warm = nc.dram_tensor("warm", [P, J * KC], u8, kind="Internal")
    # row r = p*(TILES*J) + t*J + j
    xr = x.ap().rearrange("(p t j) c -> t p (j c)", p=P, t=TILES)
    orj = out.ap().rearrange("(p t j) kc -> t p j kc", p=P, t=TILES)

    with _SplitDrainTileContext(nc) as tc:
        HK = (n_bits + 1) // 2    # planes per half-tile cast
        with (
            tc.tile_pool(name="xin", bufs=TILES) as xin,
            tc.tile_pool(name="yint", bufs=3) as yip,
            tc.tile_pool(name="st16a", bufs=5) as s16a,
            tc.tile_pool(name="st16b", bufs=5) as s16b,
        ):
            # all input DMAs first on the SP ring: they drain during the
            # compute ramp, so the steady state is pure output traffic
            xts = []
            for t in range(TILES):
                xt = xin.tile([P, JC], f32)
                nc.sync.dma_start(xt[:], xr[t])
                xts.append(xt)
            # SWDGE prewarm on both queues during the input ramp (source
            # values irrelevant, dest is scratch)
            wt = s16a.tile([P, J * HK * C], u16, tag="warm", bufs=1)
            nc.gpsimd.memset(wt[:, :64], 0)
            for q in range(2):
                wi = nc.gpsimd.dma_start(
                    warm.ap()[:, : J * HK * C], wt[:]
                )
                if q:
                    wi.ins.queue = "qPoolDynamic1"
            del wt
            for t in range(TILES):
                xt = xts[t]
                yi = yip.tile([P, JC], u16)
                # yi = floor(x * 2^n_bits) on ACT (exact, see module doc)
                nc.scalar.activation(
                    yi[:], xt[:], mybir.ActivationFunctionType.Copy,
                    bias=FLOOR_BIAS, scale=SCALE,
                )
                yiv = yi[:].rearrange("p (j c) -> p j c", j=J)
                # two half-tile stages with independent recycle: the
                # cast DMA for planes [0, HK) fires while DVE is still
                # extracting planes [HK, n)
                sta = s16a.tile([P, J * HK * C], u16)
                stb = s16b.tile([P, J * (n_bits - HK) * C], u16)
                sva = sta[:].rearrange("p (j k c) -> p j k c", j=J, k=HK)
                svb = stb[:].rearrange(
                    "p (j k c) -> p j k c", j=J, k=n_bits - HK
                )
                for k in range(n_bits):
                    dst = (
                        sva[:, :, k, :] if k < HK
                        else svb[:, :, k - HK, :]
                    )
                    nc.vector.tensor_scalar(
                        dst, yiv, n_bits - 1 - k, 1,
                        A.logical_shift_right, A.bitwise_and,
                    )
                    if k == HK - 1:
                        oi = nc.gpsimd.dma_start(
                            orj[t][:, :, : HK * C], sva
                        )
                        if t % 2:
                            oi.ins.queue = "qPoolDynamic1"
                oi = nc.gpsimd.dma_start(orj[t][:, :, HK * C :], svb)
                if (t + 1) % 2:
                    oi.ins.queue = "qPoolDynamic1"
    _nc_cache[n_bits] = nc
    return nc


def kernel(**inputs) -> np.ndarray:
    x = np.ascontiguousarray(np.asarray(inputs["x"], dtype=np.float32))
    n_bits = int(inputs["n_bits"])
    assert x.shape == (B, T, C), x.shape
    nc = _build(n_bits)
    xs = x.reshape(N_CORES, ROWS, C)
    in_maps = [{"x": xs[c]} for c in range(N_CORES)]
    res = run_bass_kernel_spmd(nc, in_maps, core_ids=list(range(N_CORES)))
    out = np.stack(
        [res.results[c]["out"] for c in range(N_CORES)], axis=0
    )  # [8, 2048, n_bits*512] u8; row r = p*(TILES*J) + t*J + j
    return out.reshape(B, T, n_bits, C).astype(np.float32)


# revision 38
# speedup vs baseline: 1.4915x; 1.1142x over previous
"""Binary successive-approximation encoder on 8 Trainium2 NeuronCores.

Full input x [16, 1024, 512] f32 -> output [16, 1024, n_bits, 512] f32.

Math: for y in [0, 1) on the 2^-23 grid (jax uniform f32), plane k
(MSB first) is bit (n_bits-1-k) of floor(y * 2^n_bits).

Pipeline, per 256-row tile (J=2 consecutive rows per partition):
  ACT  : yi = u16(round(x*2^n_bits - (0.5 - 2^(n_bits-24))))
         == floor(x*2^n_bits) EXACTLY: the bias shifts every grid point
         strictly inside a round-to-nearest window (never a tie), and
         the f32 mult/sub are exact on the grid (24-bit span).
  DVE  : plane k = (yi >> (n_bits-1-k)) & 1, one fused u16 bitvec
         tensor_scalar per plane (u16 keeps the DVE 16-bit fast path,
         0.4 ns/lane-elem measured; bitvec cannot cast to u8).
  The u16 -> u8 compaction is the bottleneck and is split over two
  parallel channels:
   - planes [0, SW): SWDGE casting DMAs u16 SBUF -> u8 HBM (only the
     software DGE can cast; its single queue sustains ~210 GB/s written
     = ~630 GB/s moved, and DmaMemcpy is hardwired to ring 0 -- a
     second SWDGE queue never receives traffic). Two half-plane casts
     per tile from independent pools for finer buffer recycle.
   - planes [SW, n): ACT copy-casts into a u8 staging tile (~1
     ns/lane-elem), drained by plain HWDGE DMAs on the SP ring. ACT
     cast for tile t is emitted AFTER the scale for tile t+1 so the
     scale (which gates DVE) is never stuck behind a cast.
The host upcasts u8 -> f32 at gather (exact: values are 0/1).

Row mapping r = p*(TILES*J) + t*J + j keeps every DMA contiguous per
partition (SWDGE descriptors are generated in software on the Q7; a
fragmented pattern multiplies their count and cost).

The first SWDGE DMA of a given size class pays a one-time ~10us ucode
cost, so one full-size prewarm DMA to a scratch tensor runs during the
input ramp.

Measured landscape (per core): DVE extraction ~31us, SWDGE channel
~5us per plane (3.15 MB moved), ACT ~1ns/elem, HBM traffic 14.5 MB.
The f32 baseline (44 MB traffic + double DVE pass) ran 124us.

Sharding: batch dim 16 -> 8 cores x 2 batches, no communication.

This walrus build allows only ONE sync wait per instruction, hence
_SplitDrainTileContext: every scheduled instruction with N>1 waits gets
N-1 preceding same-engine no-ops carrying one wait each, and the tail
drain's aggregated waits ride on SP no-ops.
"""

import numpy as np

import concourse.bass as bass
import concourse.mybir as mybir
import concourse.tile as tile
from concourse.bass_utils import run_bass_kernel_spmd

B, T, C = 16, 1024, 512
N_CORES = 8
P = 128                       # SBUF partitions
ROWS = B * T // N_CORES       # 2048 (b,t) rows per core
TILES = 8
J = ROWS // (P * TILES)       # 2 consecutive rows per partition per tile

_nc_cache: dict[int, bass.Bass] = {}


class _SplitDrainTileContext(tile.TileContext):
    """TileContext for a walrus build that rejects multi-wait instructions
    ("Too many sync wait commands", one sync wait allowed per instruction):
    every scheduled instruction with N>1 waits is preceded by N-1 same-engine
    no-ops carrying one wait each (same-engine in-order execution makes this
    equivalent), and the tail drain's aggregated waits ride on SP no-ops."""

    def _add_instruction(self, inst):
        si = inst.sync_info
        if (
            si is not None
            and si.on_wait
            and len(si.on_wait) > 1
            and inst.engine != mybir.EngineType.Unassigned
        ):
            waits = list(si.on_wait)
            si.on_wait = waits[-1:]
            for w in waits[:-1]:
                nop = mybir.InstNoOp(
                    name=self.nc.get_next_instruction_name(),
                    sync_info=mybir.SyncInfo(on_wait=[w], on_update=[]),
                    bass_nofuse=True,
                    engine=inst.engine,
                )
                super()._add_instruction(nop)
        super()._add_instruction(inst)

    def _drain_and_barrier(self, tick_clock, wait_clock):
        import bass_rust
        from concourse.vector_clock import ScopedClock

        nc = self.nc
        drain_inst = nc.sync.drain()
        wait_clock.add_sem_waits(
            drain_inst.ins, ScopedClock({None: tick_clock.global_clock})
        )
        si = drain_inst.ins.sync_info
        waits = list(si.on_wait) if si is not None else []
        if len(waits) > 1:
            si.on_wait = waits[:1]
            for w in waits[1:]:
                nop = nc.sync.nop()
                nop.ins.sync_info = bass_rust.SyncInfo(on_wait=[w], on_update=[])
        nc.all_engine_barrier()
        assert self.sems is not None
        popped = nc._tile_sem_poison_stack.pop()
        assert popped is self._sem_poison
        nc.clear_and_free_semaphores(list(self.sems.allocated().values()))
        nc.all_engine_barrier()


def _build(n_bits: int) -> bass.Bass:
    if n_bits in _nc_cache:
        return _nc_cache[n_bits]
    A = mybir.AluOpType
    f32, u16, u8 = mybir.dt.float32, mybir.dt.uint16, mybir.dt.uint8
    KC = n_bits * C
    # u16 planes + exact-floor bias both need n_bits <= 15
    assert 1 <= n_bits <= 15
    SCALE = float(2**n_bits)
    FLOOR_BIAS = -(0.5 - 2.0 ** (n_bits - 24))
    JC = J * C
    # channel split: ACT copy-casts the last AC planes, SWDGE the rest
    AC = max(1, round(n_bits * 0.3)) if n_bits >= 4 else 0
    SW = n_bits - AC
    HA = (SW + 1) // 2        # planes in the first SWDGE half-cast

    nc = bass.Bass("TRN2", target_bir_lowering=False, debug=False)
    x = nc.dram_tensor("x", [ROWS, C], f32, kind="ExternalInput")
    out = nc.dram_tensor("out", [ROWS, KC], u8, kind="ExternalOutput")
    warm = nc.dram_tensor("warm", [P, J * KC], u8, kind="Internal")
    # row r = p*(TILES*J) + t*J + j
    xr = x.ap().rearrange("(p t j) c -> t p (j c)", p=P, t=TILES)
    orj = out.ap().rearrange("(p t j) kc -> t p j kc", p=P, t=TILES)

    with _SplitDrainTileContext(nc) as tc:
        with (
            tc.tile_pool(name="xin", bufs=TILES) as xin,
            tc.tile_pool(name="yint", bufs=3) as yip,
            tc.tile_pool(name="st16a", bufs=5) as s16a,
            tc.tile_pool(name="st16b", bufs=5) as s16b,
            tc.tile_pool(name="st16c", bufs=4) as s16c,
            tc.tile_pool(name="st8", bufs=4) as s8p,
        ):
            # all input DMAs first on the SP ring: they drain during the
            # compute ramp, so the steady state is pure output traffic
            xts = []
            for t in range(TILES):
                xt = xin.tile([P, JC], f32)
                nc.sync.dma_start(xt[:], xr[t])
                xts.append(xt)
            # SWDGE prewarm during the input ramp (source values
            # irrelevant, dest is scratch)
            wt = s16a.tile([P, J * HA * C], u16, tag="warm", bufs=1)
            nc.gpsimd.memset(wt[:, :64], 0)
            nc.gpsimd.dma_start(warm.ap()[:, : J * HA * C], wt[:])
            del wt

            def scale(t):
                yi = yip.tile([P, JC], u16, name=f"yi{t}")
                nc.scalar.activation(
                    yi[:], xts[t][:], mybir.ActivationFunctionType.Copy,
                    bias=FLOOR_BIAS, scale=SCALE,
                )
                return yi

            yi = scale(0)
            deferred = None
            for t in range(TILES):
                yiv = yi[:].rearrange("p (j c) -> p j c", j=J)
                sta = s16a.tile([P, J * HA * C], u16)
                stb = s16b.tile([P, J * (SW - HA) * C], u16)
                sva = sta[:].rearrange("p (j k c) -> p j k c", j=J, k=HA)
                svb = stb[:].rearrange(
                    "p (j k c) -> p j k c", j=J, k=SW - HA
                )
                if AC:
                    stc = s16c.tile([P, J * AC * C], u16)
                    svc = stc[:].rearrange(
                        "p (j k c) -> p j k c", j=J, k=AC
                    )
                for k in range(n_bits):
                    dst = (
                        sva[:, :, k, :] if k < HA
                        else svb[:, :, k - HA, :] if k < SW
                        else svc[:, :, k - SW, :]
                    )
                    nc.vector.tensor_scalar(
                        dst, yiv, n_bits - 1 - k, 1,
                        A.logical_shift_right, A.bitwise_and,
                    )
                    if k == HA - 1:
                        nc.gpsimd.dma_start(orj[t][:, :, : HA * C], sva)
                    elif k == SW - 1:
                        nc.gpsimd.dma_start(
                            orj[t][:, :, HA * C : SW * C], svb
                        )
                # next tile's scale BEFORE this tile's ACT cast: the
                # scale gates DVE, the cast does not
                if t + 1 < TILES:
                    yi = scale(t + 1)
                if deferred is not None:
                    dsvc, dt_ = deferred
                    s8 = s8p.tile([P, J * AC * C], u8)
                    s8v = s8[:].rearrange(
                        "p (j k c) -> p j k c", j=J, k=AC
                    )
                    nc.scalar.copy(s8v, dsvc)
                    nc.sync.dma_start(orj[dt_][:, :, SW * C :], s8v)
                deferred = (svc, t) if AC else None
            if deferred is not None:
                dsvc, dt_ = deferred
                s8 = s8p.tile([P, J * AC * C], u8)
                s8v = s8[:].rearrange("p (j k c) -> p j k c", j=J, k=AC)
                nc.scalar.copy(s8v, dsvc)
                nc.sync.dma_start(orj[dt_][:, :, SW * C :], s8v)
    _nc_cache[n_bits] = nc
    return nc


def kernel(**inputs) -> np.ndarray:
    x = np.ascontiguousarray(np.asarray(inputs["x"], dtype=np.float32))
    n_bits = int(inputs["n_bits"])
    assert x.shape == (B, T, C), x.shape
    nc = _build(n_bits)
    xs = x.reshape(N_CORES, ROWS, C)
    in_maps = [{"x": xs[c]} for c in range(N_CORES)]
    res = run_bass_kernel_spmd(nc, in_maps, core_ids=list(range(N_CORES)))
    out = np.stack(
        [res.results[c]["out"] for c in range(N_CORES)], axis=0
    )  # [8, 2048, n_bits*512] u8; row r = p*(TILES*J) + t*J + j
    return out.reshape(B, T, n_bits, C).astype(np.float32)
